# revision 1
# baseline (speedup 1.0000x reference)
"""Trainium2 Bass kernel: batched affine bilinear sampling (spatial transformer).

Full inputs: images [32, 512, 512, 3] f32, theta [32, 2, 3] f32.
Data parallel over batch: 8 NeuronCores x 4 images each; per core, each image
is processed by one SPMD launch of a 4-block NEFF (one shared quad-image
staging pass serves all four 128-row output blocks of the image).

Device algorithm per launch:
  1. Stage the image into a DRAM "quad image"
     imgQ[y*512+x] = [img[y,x,:], img[y,x+1,:], img[y+1,x,:], img[y+1,x+1,:]]
     so the 4 bilinear neighbors of any sample sit in one 48B row.
  2. Compute per-pixel sample coordinates / lerp weights on DVE+ACT from theta
     (exact floor via 2^23 magic-round + compare; exact clamp semantics of the
     reference, including its zero-weight right/bottom overflow edges).
  3. Gather one 48B quad per output pixel via indirect DMA, 128 offsets (one
     per partition) per instruction - the only indirect-DMA configuration that
     is correct on this hardware stack (multi-offset lowering corrupts data;
     dma_gather requires 256B elements + int16 indices).
  4. Blend and store per 128-column quarter so DVE work and output DMA overlap
     the remaining gather stream via sub-tile dependencies.
"""

import sys
from contextlib import ExitStack

for _p in ("/opt/trn_rl_repo",):
    if _p not in sys.path:
        sys.path.append(_p)

import numpy as np

import concourse.bacc as bacc
import concourse.bass as bass
import concourse.tile as tile
from concourse import mybir
from concourse.bass import IndirectOffsetOnAxis
from concourse.bass_utils import run_bass_kernel_spmd

F32 = mybir.dt.float32
I32 = mybir.dt.int32
OP = mybir.AluOpType
ACTF = mybir.ActivationFunctionType

H = W = 512
P = 128
NBLK = H // P
MAGIC = float(2 ** 23)
N_CORES = 8
BPL = 4  # blocks per launch
def _body(ctx: ExitStack, tc: "tile.TileContext", imgs: bass.AP,
          theta: bass.AP, bb: bass.AP, out: bass.AP):
    nc = tc.nc

    const_pool = ctx.enter_context(tc.tile_pool(name="const", bufs=1))
    stage_pool = ctx.enter_context(tc.tile_pool(name="stage", bufs=2))
    pairs_pool = ctx.enter_context(tc.tile_pool(name="pairs", bufs=1))
    tiny_pool = ctx.enter_context(tc.tile_pool(name="tiny", bufs=2))
    coord_pool = ctx.enter_context(tc.tile_pool(name="coord", bufs=2))
    quad_pool = ctx.enter_context(tc.tile_pool(name="quad", bufs=1))
    blend_pool = ctx.enter_context(tc.tile_pool(name="blend", bufs=2))
    dram_pool = ctx.enter_context(tc.tile_pool(name="dramp", bufs=1, space="DRAM"))

    th = const_pool.tile([P, 6], F32)
    nc.sync.dma_start(out=th, in_=theta.unsqueeze(0).to_broadcast([P, 6]))
    bbs = const_pool.tile([P, BPL], F32)
    nc.sync.dma_start(out=bbs, in_=bb.unsqueeze(0).to_broadcast([P, BPL]))

    iota_row_i = const_pool.tile([P, W], I32)
    nc.gpsimd.iota(iota_row_i, [[1, W]], base=0, channel_multiplier=0)
    gx = const_pool.tile([P, W], F32)
    nc.vector.tensor_copy(out=gx, in_=iota_row_i)
    nc.vector.tensor_scalar(out=gx, in0=gx, scalar1=2.0 / 511.0, scalar2=-1.0,
                            op0=OP.mult, op1=OP.add)

    iota_col_i = const_pool.tile([P, 1], I32)
    nc.gpsimd.iota(iota_col_i, [[0, 1]], base=0, channel_multiplier=1)
    iotacf = const_pool.tile([P, 1], F32)
    nc.vector.tensor_copy(out=iotacf, in_=iota_col_i)

    a_ = th[:, 0:1]; b_ = th[:, 1:2]; c_ = th[:, 2:3]
    d_ = th[:, 3:4]; e_ = th[:, 4:5]; f_ = th[:, 5:6]

    zero12 = const_pool.tile([1, 12], F32)
    nc.vector.memset(zero12, 0.0)
    imgQ = dram_pool.tile([H * W + 1, 12], F32, name="imgQ")
    nc.sync.dma_start(out=imgQ[H * W:H * W + 1, :], in_=zero12)
    imgs_flat = imgs.rearrange("k h w c -> k (h w c)")
    for blk in range(NBLK):
        r0 = blk * P
        loadAB = stage_pool.tile([P, 2, (W + 1) * 3], F32)
        src = bass.AP(
            tensor=imgs_flat.tensor,
            offset=imgs_flat.offset + r0 * W * 3,
            ap=[[W * 3, P], [W * 3, 2], [1, (W + 1) * 3]],
        )
        nc.scalar.dma_start(out=loadAB, in_=src)
        lab = loadAB.rearrange("p j (w c) -> p j w c", c=3)
        pairs = pairs_pool.tile([P, W, 4, 3], F32)
        nc.scalar.activation(out=pairs[:, :, 0, :], in_=lab[:, 0, 0:W, :], func=ACTF.Copy)
        nc.vector.tensor_copy(out=pairs[:, :, 1, :], in_=lab[:, 0, 1:W + 1, :])
        nc.scalar.activation(out=pairs[:, :, 2, :], in_=lab[:, 1, 0:W, :], func=ACTF.Copy)
        nc.vector.tensor_copy(out=pairs[:, :, 3, :], in_=lab[:, 1, 1:W + 1, :])
        nc.sync.dma_start(
            out=imgQ[r0 * W:(r0 + P) * W, :].rearrange("(p n) c -> p (n c)", p=P),
            in_=pairs.rearrange("p w j c -> p (w j c)"))

    A256 = tiny_pool.tile([P, 1], F32, name="A256")
    nc.vector.tensor_scalar_mul(A256, a_, 256.0)
    D256 = tiny_pool.tile([P, 1], F32, name="D256")
    nc.vector.tensor_scalar_mul(D256, d_, 256.0)
    c1x = tiny_pool.tile([P, 1], F32, name="c1x")
    nc.vector.tensor_scalar(out=c1x, in0=c_, scalar1=1.0, scalar2=256.0,
                            op0=OP.add, op1=OP.mult)
    c1y = tiny_pool.tile([P, 1], F32, name="c1y")
    nc.vector.tensor_scalar(out=c1y, in0=f_, scalar1=1.0, scalar2=256.0,
                            op0=OP.add, op1=OP.mult)
    xa = tiny_pool.tile([P, W], F32, name="xa")
    nc.vector.tensor_scalar(out=xa, in0=gx, scalar1=A256, scalar2=None, op0=OP.mult)
    ya = tiny_pool.tile([P, W], F32, name="ya")
    nc.vector.tensor_scalar(out=ya, in0=gx, scalar1=D256, scalar2=None, op0=OP.mult)

    for q in range(BPL):
        gyb = tiny_pool.tile([P, 1], F32, name="gyb")
        nc.vector.tensor_scalar(out=gyb, in0=iotacf, scalar1=512.0 / 511.0,
                                scalar2=bbs[:, q:q + 1], op0=OP.mult, op1=OP.add)
        sx = tiny_pool.tile([P, 1], F32, name="sx")
        nc.vector.tensor_scalar(out=sx, in0=gyb, scalar1=b_, scalar2=c1x,
                                op0=OP.mult, op1=OP.add)
        sy = tiny_pool.tile([P, 1], F32, name="sy")
        nc.vector.tensor_scalar(out=sy, in0=gyb, scalar1=e_, scalar2=c1y,
                                op0=OP.mult, op1=OP.add)

        def coord_side(arow, scol, tag):
            v = coord_pool.tile([P, W], F32, name=f"v{tag}")
            nc.vector.tensor_scalar(out=v, in0=arow, scalar1=scol, scalar2=None,
                                    op0=OP.add)
            r = coord_pool.tile([P, W], F32, name=f"r{tag}")
            nc.scalar.activation(out=r, in_=v, func=ACTF.Copy, bias=MAGIC)
            nc.scalar.activation(out=r, in_=r, func=ACTF.Copy, bias=-MAGIC)
            g = coord_pool.tile([P, W], F32, name=f"g{tag}")
            nc.vector.tensor_tensor(out=g, in0=r, in1=v, op=OP.is_gt)
            nc.vector.tensor_sub(r, r, g)
            nc.vector.tensor_scalar(out=r, in0=r, scalar1=0.0, scalar2=511.0,
                                    op0=OP.max, op1=OP.min)
            p1 = coord_pool.tile([P, W], F32, name=f"p1{tag}")
            nc.vector.tensor_scalar(out=p1, in0=r, scalar1=1.0, scalar2=511.0,
                                    op0=OP.add, op1=OP.min)
            nc.vector.tensor_scalar(out=v, in0=v, scalar1=0.0, scalar2=511.0,
                                    op0=OP.max, op1=OP.min)
            nc.vector.tensor_sub(p1, p1, v)
            nc.vector.tensor_sub(v, v, r)
            return p1, v, r

        u0, u1, x0f = coord_side(xa, sx, "x")
        v0, v1, y0f = coord_side(ya, sy, "y")

        idxf = coord_pool.tile([P, W], F32)
        nc.vector.tensor_scalar(out=idxf, in0=y0f, scalar1=512.0, scalar2=None,
                                op0=OP.mult)
        nc.vector.tensor_add(idxf, idxf, x0f)
        idxi = coord_pool.tile([P, W], I32)
        nc.vector.tensor_copy(out=idxi, in_=idxf)

        quad = quad_pool.tile([P, W, 12], F32, name="quad")
        QW = W // 4
        for s in range(4):
            for ox in range(s * QW, (s + 1) * QW):
                nc.gpsimd.indirect_dma_start(
                    out=quad[:, ox, :],
                    out_offset=None,
                    in_=imgQ[:, :],
                    in_offset=IndirectOffsetOnAxis(ap=idxi[:, ox:ox + 1], axis=0),
                )
            sl = slice(s * QW, (s + 1) * QW)
            q4 = quad[:, sl, :].rearrange("p w (jk c) -> p w jk c", c=3)
            tmp6 = blend_pool.tile([P, QW, 2, 3], F32, name="tmp6")
            u0b = u0[:, sl].unsqueeze(2).unsqueeze(3).to_broadcast([P, QW, 2, 3])
            u1b = u1[:, sl].unsqueeze(2).unsqueeze(3).to_broadcast([P, QW, 2, 3])
            nc.vector.tensor_mul(tmp6, q4[:, :, 1:4:2, :], u1b)
            nc.vector.tensor_mul(q4[:, :, 0:4:2, :], q4[:, :, 0:4:2, :], u0b)
            nc.vector.tensor_add(q4[:, :, 0:4:2, :], q4[:, :, 0:4:2, :], tmp6)
            v0b = v0[:, sl].unsqueeze(2).to_broadcast([P, QW, 3])
            v1b = v1[:, sl].unsqueeze(2).to_broadcast([P, QW, 3])
            outt = blend_pool.tile([P, QW, 3], F32, name="outt")
            tmp3 = blend_pool.tile([P, QW, 3], F32, name="tmp3")
            nc.vector.tensor_mul(outt, q4[:, :, 0, :], v0b)
            nc.vector.tensor_mul(tmp3, q4[:, :, 2, :], v1b)
            nc.vector.tensor_add(outt, outt, tmp3)
            nc.sync.dma_start(out=out[q, :, sl, :], in_=outt)



def build_kernel2(num_devices: int = N_CORES):
    nc = bacc.Bacc("TRN2", target_bir_lowering=False, debug=False,
                   num_devices=num_devices)
    imgs = nc.dram_tensor("imgs", [1, H + 2, W, 3], F32, kind="ExternalInput")
    theta = nc.dram_tensor("theta", [6], F32, kind="ExternalInput")
    bb = nc.dram_tensor("bb", [BPL], F32, kind="ExternalInput")
    out = nc.dram_tensor("out", [BPL, P, W, 3], F32, kind="ExternalOutput")
    with tile.TileContext(nc) as tc:
        with ExitStack() as ctx:
            _body(ctx, tc, imgs.ap(), theta.ap(), bb.ap(), out.ap())
    nc.compile()
    return nc


_NC_CACHE = {}


def run_kernel_spmd(images: np.ndarray, theta: np.ndarray, trace: bool = False):
    B = images.shape[0]
    per = B // N_CORES
    if "k2" not in _NC_CACHE:
        _NC_CACHE["k2"] = build_kernel2(N_CORES)
    nc = _NC_CACHE["k2"]

    out = np.zeros((B, H, W, 3), np.float32)
    slabs = []
    for c in range(N_CORES):
        s = np.zeros((per, H + 2, W, 3), np.float32)
        s[:, :H] = images[c * per:(c + 1) * per]
        slabs.append(s)

    last_res = None
    for k in range(per):
        for half in range(NBLK // BPL):
            bbv = np.array(
                [128.0 * (BPL * half + q) * (512.0 / 511.0) - 256.0
                 for q in range(BPL)], np.float32)
            in_maps = []
            for c in range(N_CORES):
                in_maps.append({
                    "imgs": slabs[c][k:k + 1],
                    "theta": np.ascontiguousarray(
                        theta[c * per + k].reshape(-1)).astype(np.float32),
                    "bb": bbv,
                })
            res = run_bass_kernel_spmd(nc, in_maps, core_ids=list(range(N_CORES)),
                                       trace=trace)
            last_res = res
            r0 = half * BPL * P
            for c in range(N_CORES):
                out[c * per + k, r0:r0 + BPL * P] = \
                    res.results[c]["out"].reshape(BPL * P, W, 3)
    return out, last_res


def kernel(images: np.ndarray, theta: np.ndarray) -> np.ndarray:
    images = np.ascontiguousarray(np.asarray(images), dtype=np.float32)
    theta = np.asarray(theta).astype(np.float32)
    out, _ = run_kernel_spmd(images, theta, trace=False)
    return out

